# revision 3
# baseline (speedup 1.0000x reference)
"""CombinedSparsity (spatial max-pool + lifetime top-k + max-unpool) on 8 TRN2 cores.

Strategy: shard the 128 channels across 8 cores (16 each). Per (b, c) map the
output is all zeros except (possibly) one element: the map's max, written back
at its argmax position, kept only if that max is among the top-6 over the batch
for its channel. So instead of writing a dense 268MB output, each core:
  1. streams its 33.5MB shard through one DVE max-reduce per channel,
  2. finds the per-channel top-8 batch entries with one InstMax/InstMaxIndex
     on the transposed pooled matrix,
  3. re-gathers only the candidate maps (top-6 x channels) via indirect
     DMA and locates each map's argmax by value-matching (InstMaxIndex),
  4. scatters the surviving values (6 per channel) as single f32 elements
     with an offset-bounds-checked indirect DMA; everything else stays zero
     because PJRT output buffers are donated zero-filled.

The 16 channels are processed as several pipelined units so each unit's
top-k/gather/scatter tail overlaps the next unit's load stream. The only
Vector-engine ops in a tail are max8/find8/FIND_INDEX8; all small
copies/casts/arith run on GpSimd (and the PSUM drain on Scalar) so the tail's
serial chain never queues behind the next unit's 4.3us streaming reduces.
"""
import numpy as np

import concourse.bass as bass
import concourse.bacc as bacc
import concourse.tile as tile
from concourse import mybir
from concourse.bass_utils import run_bass_kernel_spmd
from concourse.masks import make_identity

B = 128
C_FULL = 128
H = 64
W = 64
HW = H * W
N_CORES = 8
CSH = C_FULL // N_CORES      # channels per core
K = 6                        # lifetime top-k
F32 = mybir.dt.float32

_nc_cache = None


def _build():
    global _nc_cache
    if _nc_cache is not None:
        return _nc_cache

    nc = bacc.Bacc("TRN2", target_bir_lowering=False, debug=False)
    x = nc.dram_tensor("x", [B, CSH, HW], F32, kind="ExternalInput")
    y = nc.dram_tensor("y", [B, CSH, HW], F32, kind="ExternalOutput")
    x_flat = x.rearrange("b c h -> (b c) h")
    y_elem = y.rearrange("b c h -> (b c h)")[:, None]
    n_elem = B * CSH * HW

    units = [
        (0, 6),
        (6, 12),
        (12, 16),
    ]

    with tile.TileContext(nc) as tc:
        with (
            tc.tile_pool(name="const", bufs=1) as cp,
            tc.tile_pool(name="gxp", bufs=7) as gxp,
            tc.tile_pool(name="small", bufs=1) as sp,
            tc.tile_pool(name="ps", bufs=1, space="PSUM") as pp,
        ):
            ident0 = cp.tile([B, B], F32)
            make_identity(nc, ident0[:])
            # keep matmul inputs single-producer-engine (DVE)
            ident = cp.tile([B, B], F32)
            nc.vector.tensor_copy(out=ident[:], in_=ident0[:])

            def emit_unit(u, c_lo, c_hi):
                ncha = c_hi - c_lo
                nsurv = ncha * K

                # loads + per-(b,c) max over HW, one channel per group
                pooled = sp.tile([B, ncha], F32, name=f"pooled{u}")
                for ci in range(ncha):
                    c0 = c_lo + ci
                    gx = gxp.tile([B, HW], F32, tag="gx")
                    nc.sync.dma_start(out=gx[:], in_=x[:, c0, :])
                    nc.vector.tensor_reduce(
                        out=pooled[:, ci:ci + 1],
                        in_=gx[:],
                        axis=mybir.AxisListType.X,
                        op=mybir.AluOpType.max,
                    )

                # per-channel top-8 over the batch
                pooled_t_ps = pp.tile([ncha, B], F32, name=f"ptps{u}")
                nc.tensor.transpose(
                    out=pooled_t_ps[:], in_=pooled[:], identity=ident[:]
                )
                pooled_t = sp.tile([ncha, B], F32, name=f"pt{u}")
                nc.scalar.copy(out=pooled_t[:], in_=pooled_t_ps[:])

                pt8 = sp.tile([ncha, 8], F32, name=f"pt8{u}")
                nc.vector.max(out=pt8[:], in_=pooled_t[:])
                pi8 = sp.tile([ncha, 8], mybir.dt.uint32, name=f"pi8{u}")
                nc.vector.max_index(
                    out=pi8[:], in_max=pt8[:], in_values=pooled_t[:]
                )

                # trio[c, j] = (row, value, row_offset) interleaved so a single
                # SBUF->SBUF DMA compacts the j<6 survivor slots.
                pi8f = sp.tile([ncha, 8], F32, name=f"pi8f{u}")
                nc.gpsimd.tensor_copy(out=pi8f[:], in_=pi8[:])
                c_col_i = sp.tile([ncha, 1], mybir.dt.int32, name=f"cci{u}")
                nc.gpsimd.iota(
                    c_col_i[:], pattern=[[1, 1]], base=c_lo, channel_multiplier=1
                )
                c_col = sp.tile([ncha, 1], F32, name=f"cc{u}")
                nc.gpsimd.tensor_copy(out=c_col[:], in_=c_col_i[:])
                trio = sp.tile([ncha, 8, 3], F32, name=f"trio{u}")
                # trio[..0] = r8 = b_idx*CSH + c  (f32-exact DRAM row)
                nc.gpsimd.tensor_scalar(
                    out=trio[:, :, 0], in0=pi8f[:], scalar1=float(CSH),
                    scalar2=c_col[:, 0:1],
                    op0=mybir.AluOpType.mult, op1=mybir.AluOpType.add,
                )
                # trio[..1] = pooled value
                nc.gpsimd.tensor_copy(out=trio[:, :, 1], in_=pt8[:])
                # trio[..2] = r8 * HW  (element offset of row start)
                nc.gpsimd.tensor_scalar(
                    out=trio[:, :, 2], in0=trio[:, :, 0], scalar1=float(HW),
                    scalar2=None, op0=mybir.AluOpType.mult,
                )

                # compact the j<6 survivor slots: [ncha,6,3] -> [nsurv,3]
                compact = sp.tile([nsurv, 3], F32, name=f"cpk{u}")
                nc.gpsimd.dma_start(out=compact[:], in_=trio[:, 0:K, :])

                compact_r_i = sp.tile([nsurv, 1], mybir.dt.int32, name=f"cri{u}")
                nc.gpsimd.tensor_copy(out=compact_r_i[:], in_=compact[:, 0:1])

                # gather survivor maps
                cx = sp.tile([nsurv, HW], F32, name=f"cx{u}", tag="cx")
                nc.gpsimd.indirect_dma_start(
                    out=cx[:], out_offset=None,
                    in_=x_flat[:],
                    in_offset=bass.IndirectOffsetOnAxis(
                        ap=compact_r_i[:, 0:1], axis=0
                    ),
                )
                v8 = sp.tile([nsurv, 8], F32, name=f"v8{u}")
                nc.gpsimd.tensor_copy(
                    out=v8[:], in_=compact[:, 1:2].to_broadcast([nsurv, 8])
                )
                hw8 = sp.tile([nsurv, 8], mybir.dt.uint32, name=f"hw8{u}")
                nc.vector.max_index(out=hw8[:], in_max=v8[:], in_values=cx[:])

                hwf = sp.tile([nsurv, 1], F32, name=f"hwf{u}")
                nc.gpsimd.tensor_copy(out=hwf[:], in_=hw8[:, 0:1])
                coff_f = sp.tile([nsurv, 1], F32, name=f"cof{u}")
                nc.gpsimd.tensor_tensor(
                    out=coff_f[:], in0=compact[:, 2:3], in1=hwf[:],
                    op=mybir.AluOpType.add,
                )
                coff_i = sp.tile([nsurv, 1], mybir.dt.int32, name=f"coi{u}")
                nc.gpsimd.tensor_copy(out=coff_i[:], in_=coff_f[:])

                nc.gpsimd.indirect_dma_start(
                    out=y_elem[:],
                    out_offset=bass.IndirectOffsetOnAxis(
                        ap=coff_i[:, 0:1], axis=0
                    ),
                    in_=compact[:, 1:2],
                    in_offset=None,
                    bounds_check=n_elem - 1,
                    oob_is_err=False,
                )

            for u, (c_lo, c_hi) in enumerate(units):
                emit_unit(u, c_lo, c_hi)

    nc.finalize()
    _nc_cache = nc
    return nc


def _install_profile_hook():
    """Inject the antenv.axon_hooks shim so trace=True captures NTFFs."""
    import sys
    import types

    if "antenv.axon_hooks" in sys.modules:
        return
    import antenv
    import trn_agent_boot.trn_boot as tb

    mod = types.ModuleType("antenv.axon_hooks")
    mod._hook = tb._ntff_profile_via_ctypes("/opt/axon/libaxon_pjrt.so")
    mod.get_axon_ntff_profile_hook = lambda: mod._hook
    mod.set_axon_ntff_profile_hook = lambda h: setattr(mod, "_hook", h)
    sys.modules["antenv.axon_hooks"] = mod
    antenv.axon_hooks = mod

    # no S3 in this container — keep artifacts local
    import concourse.bass_utils as bu

    bu.upload_artifacts = lambda tmpdir: tmpdir


def run(activations, trace=False):
    if trace:
        _install_profile_hook()
    act = np.asarray(activations)
    assert act.shape == (B, C_FULL, H, W), act.shape
    act = act.astype(np.float32, copy=False)
    nc = _build()
    in_maps = [
        {"x": np.ascontiguousarray(act[:, i * CSH:(i + 1) * CSH]).reshape(B, CSH, HW)}
        for i in range(N_CORES)
    ]
    res = run_bass_kernel_spmd(
        nc, in_maps, core_ids=list(range(N_CORES)), trace=trace
    )
    out = np.concatenate(
        [r["y"].reshape(B, CSH, H, W) for r in res.results], axis=1
    )
    return out, res


def kernel(activations):
    out, _ = run(activations, trace=False)
    return out


# revision 5
# speedup vs baseline: 1.0030x; 1.0030x over previous
"""CombinedSparsity (spatial max-pool + lifetime top-k + max-unpool) on 8 TRN2 cores.

Shard the 128 channels across 8 cores (16 each). Per (b, c) map the output is
all zeros except (possibly) one element: the map's max, written back at its
argmax position, kept only if that max is among the top-6 over the batch for
its channel. Output buffers are donated zero-filled, so each core only writes
the surviving elements.

Per core, channels are processed in three pipelined units:
  - units A (ch 0-10) and B (ch 11-14) use the slot path: InstMax/InstMaxIndex
    top-8 on the transposed pooled matrix, compact the 6 survivor slots via an
    SBUF->SBUF DMA, indirect-gather the survivor maps, re-locate each argmax by
    value-matching (InstMaxIndex), scatter single elements. Their FIND passes
    are placed late in the Vector stream so the mid-stream DMA round trips
    never stall the streaming reduces.
  - unit D (ch 15, the last one streamed) uses a threshold path that shortens
    the post-stream serial chain: its reduce keeps 64 chunk maxima, the max's
    chunk index comes from one small FIND, the winning 64-element chunk is
    re-gathered, masked to {max at argmax, 0 elsewhere} by three Scalar
    activation passes, and scattered as a 256B row per surviving batch entry.
    Survivors are selected by value threshold (6th-largest, broadcast via PE
    matmul) with exact tie handling: keep pooled > thr, or pooled == thr and
    b <= b_thr where b_thr is slot 5 of the progressive InstMaxIndex.
"""
import numpy as np

import concourse.bass as bass
import concourse.bacc as bacc
import concourse.tile as tile
from concourse import mybir
from concourse.bass_utils import run_bass_kernel_spmd
from concourse.masks import make_identity

B = 128
C_FULL = 128
H = 64
W = 64
HW = H * W
N_CORES = 8
CSH = C_FULL // N_CORES      # channels per core
K = 6                        # lifetime top-k
NCHUNK = 64                  # chunks per map in the threshold path
CHW = HW // NCHUNK           # elements per chunk
F32 = mybir.dt.float32
I32 = mybir.dt.int32
U32 = mybir.dt.uint32
BIG = float(1 << 23)         # OOB mask offset, f32-exact

_nc_cache = None


def _build():
    global _nc_cache
    if _nc_cache is not None:
        return _nc_cache

    nc = bacc.Bacc("TRN2", target_bir_lowering=False, debug=False)
    x = nc.dram_tensor("x", [B, CSH, HW], F32, kind="ExternalInput")
    y = nc.dram_tensor("y", [B, CSH, HW], F32, kind="ExternalOutput")
    x_flat = x.rearrange("b c h -> (b c) h")
    y_elem = y.rearrange("b c h -> (b c h)")[:, None]
    x_rows = x.rearrange("b c (k j) -> (b c k) j", j=CHW)
    y_rows = y.rearrange("b c (k j) -> (b c k) j", j=CHW)
    n_elem = B * CSH * HW
    n_rows = B * CSH * NCHUNK

    UNITS = [(0, 11), (11, 15)]  # slot-path units
    CD = 15                      # threshold-path channel

    with tile.TileContext(nc) as tc:
        with (
            tc.tile_pool(name="const", bufs=1) as cp,
            tc.tile_pool(name="gxp", bufs=5) as gxp,
            tc.tile_pool(name="small", bufs=1) as sp,
            tc.tile_pool(name="ps", bufs=2, space="PSUM") as pp,
        ):
            ident0 = cp.tile([B, B], F32)
            make_identity(nc, ident0[:])
            ident = cp.tile([B, B], F32)
            nc.vector.tensor_copy(out=ident[:], in_=ident0[:])
            ones_row_i = cp.tile([1, B], I32)
            nc.gpsimd.iota(ones_row_i[:], pattern=[[0, B]], base=1,
                           channel_multiplier=0)
            ones_row = cp.tile([1, B], F32)
            nc.gpsimd.tensor_copy(out=ones_row[:], in_=ones_row_i[:])
            iota_b_i = cp.tile([B, 1], I32)
            nc.gpsimd.iota(iota_b_i[:], pattern=[[1, 1]], base=0,
                           channel_multiplier=1)
            iota_b = cp.tile([B, 1], F32)
            nc.gpsimd.tensor_copy(out=iota_b[:], in_=iota_b_i[:])
            # b*CSH*NCHUNK: row index of (b, c=0, chunk=0) in x_rows/y_rows
            brow = cp.tile([B, 1], F32)
            nc.gpsimd.tensor_scalar(
                out=brow[:], in0=iota_b[:], scalar1=float(CSH * NCHUNK),
                scalar2=None, op0=mybir.AluOpType.mult,
            )

            pooled = sp.tile([B, CSH], F32, name="pooled")

            def emit_stream_channel(c):
                gx = gxp.tile([B, HW], F32, tag="gx")
                nc.sync.dma_start(out=gx[:], in_=x[:, c, :])
                nc.vector.tensor_reduce(
                    out=pooled[:, c:c + 1], in_=gx[:],
                    axis=mybir.AxisListType.X, op=mybir.AluOpType.max,
                )

            def emit_topk(u, c_lo, c_hi):
                """transpose + max8/find8 for channels [c_lo, c_hi)."""
                ncha = c_hi - c_lo
                pooled_t_ps = pp.tile([ncha, B], F32, name=f"ptps{u}")
                nc.tensor.transpose(
                    out=pooled_t_ps[:], in_=pooled[:, c_lo:c_hi],
                    identity=ident[:],
                )
                pooled_t = sp.tile([ncha, B], F32, name=f"pt{u}")
                nc.scalar.copy(out=pooled_t[:], in_=pooled_t_ps[:])
                pt8 = sp.tile([ncha, 8], F32, name=f"pt8{u}")
                nc.vector.max(out=pt8[:], in_=pooled_t[:])
                pi8 = sp.tile([ncha, 8], U32, name=f"pi8{u}")
                nc.vector.max_index(
                    out=pi8[:], in_max=pt8[:], in_values=pooled_t[:]
                )
                return pt8, pi8

            def emit_slot_pre(u, c_lo, c_hi, pt8, pi8):
                """gpsimd chain: trio, compact, survivor-map gather."""
                ncha = c_hi - c_lo
                nsurv = ncha * K
                pi8f = sp.tile([ncha, 8], F32, name=f"pi8f{u}")
                nc.gpsimd.tensor_copy(out=pi8f[:], in_=pi8[:])
                c_col_i = sp.tile([ncha, 1], I32, name=f"cci{u}")
                nc.gpsimd.iota(
                    c_col_i[:], pattern=[[1, 1]], base=c_lo,
                    channel_multiplier=1,
                )
                c_col = sp.tile([ncha, 1], F32, name=f"cc{u}")
                nc.gpsimd.tensor_copy(out=c_col[:], in_=c_col_i[:])
                trio = sp.tile([ncha, 8, 3], F32, name=f"trio{u}")
                nc.gpsimd.tensor_scalar(
                    out=trio[:, :, 0], in0=pi8f[:], scalar1=float(CSH),
                    scalar2=c_col[:, 0:1],
                    op0=mybir.AluOpType.mult, op1=mybir.AluOpType.add,
                )
                nc.gpsimd.tensor_copy(out=trio[:, :, 1], in_=pt8[:])
                nc.gpsimd.tensor_scalar(
                    out=trio[:, :, 2], in0=trio[:, :, 0], scalar1=float(HW),
                    scalar2=None, op0=mybir.AluOpType.mult,
                )
                compact = sp.tile([nsurv, 3], F32, name=f"cpk{u}")
                nc.gpsimd.dma_start(out=compact[:], in_=trio[:, 0:K, :])
                compact_r_i = sp.tile([nsurv, 1], I32, name=f"cri{u}")
                nc.gpsimd.tensor_copy(out=compact_r_i[:], in_=compact[:, 0:1])
                cx = sp.tile([nsurv, HW], F32, name=f"cx{u}", tag="cx")
                nc.gpsimd.indirect_dma_start(
                    out=cx[:], out_offset=None, in_=x_flat[:],
                    in_offset=bass.IndirectOffsetOnAxis(
                        ap=compact_r_i[:, 0:1], axis=0
                    ),
                )
                v8 = sp.tile([nsurv, 8], F32, name=f"v8{u}")
                nc.gpsimd.tensor_copy(
                    out=v8[:], in_=compact[:, 1:2].to_broadcast([nsurv, 8])
                )
                return compact, cx, v8

            def emit_slot_find(u, c_lo, c_hi, compact, cx, v8):
                """the big value-match FIND + element scatter."""
                nsurv = (c_hi - c_lo) * K
                hw8 = sp.tile([nsurv, 8], U32, name=f"hw8{u}")
                nc.vector.max_index(out=hw8[:], in_max=v8[:], in_values=cx[:])
                hwf = sp.tile([nsurv, 1], F32, name=f"hwf{u}")
                nc.gpsimd.tensor_copy(out=hwf[:], in_=hw8[:, 0:1])
                coff_f = sp.tile([nsurv, 1], F32, name=f"cof{u}")
                nc.gpsimd.tensor_tensor(
                    out=coff_f[:], in0=compact[:, 2:3], in1=hwf[:],
                    op=mybir.AluOpType.add,
                )
                coff_i = sp.tile([nsurv, 1], I32, name=f"coi{u}")
                nc.gpsimd.tensor_copy(out=coff_i[:], in_=coff_f[:])
                nc.gpsimd.indirect_dma_start(
                    out=y_elem[:],
                    out_offset=bass.IndirectOffsetOnAxis(
                        ap=coff_i[:, 0:1], axis=0
                    ),
                    in_=compact[:, 1:2], in_offset=None,
                    bounds_check=n_elem - 1, oob_is_err=False,
                )

            # ---- unit A: stream + slot path -------------------------------
            a_lo, a_hi = UNITS[0]
            for c in range(a_lo, a_hi):
                emit_stream_channel(c)
            pt8a, pi8a = emit_topk(0, a_lo, a_hi)
            cpa, cxa, v8a = emit_slot_pre(0, a_lo, a_hi, pt8a, pi8a)

            # ---- unit B: stream; A's FIND goes mid-B ----------------------
            b_lo, b_hi = UNITS[1]
            for c in range(b_lo, b_hi):
                emit_stream_channel(c)
                if c == b_hi - 2:
                    # A's gather is ~15us old by the time the scheduler gets
                    # here; the FIND won't stall the remaining reduces.
                    emit_slot_find(0, a_lo, a_hi, cpa, cxa, v8a)
            pt8b, pi8b = emit_topk(1, b_lo, b_hi)
            cpb, cxb, v8b = emit_slot_pre(1, b_lo, b_hi, pt8b, pi8b)

            # ---- unit D: last channel, threshold path ---------------------
            gx = gxp.tile([B, HW], F32, tag="gx")
            nc.sync.dma_start(out=gx[:], in_=x[:, CD, :])
            pm = sp.tile([B, NCHUNK], F32, name="pmD")
            nc.vector.tensor_reduce(
                out=pm[:], in_=gx[:].rearrange("p (k j) -> p k j", k=NCHUNK),
                axis=mybir.AxisListType.X, op=mybir.AluOpType.max,
            )
            nc.vector.tensor_reduce(
                out=pooled[:, CD:CD + 1], in_=pm[:],
                axis=mybir.AxisListType.X, op=mybir.AluOpType.max,
            )
            pbc8 = sp.tile([B, 8], F32, name="pbc8D")
            nc.gpsimd.tensor_copy(
                out=pbc8[:], in_=pooled[:, CD:CD + 1].to_broadcast([B, 8])
            )
            cm8 = sp.tile([B, 8], U32, name="cm8D")
            nc.vector.max_index(out=cm8[:], in_max=pbc8[:], in_values=pm[:])

            pt8d, pi8d = emit_topk(2, CD, CD + 1)

            # B's FIND after D's small Vector ops so they aren't blocked
            emit_slot_find(1, b_lo, b_hi, cpb, cxb, v8b)

            # D gpsimd chain: chunk row, gather
            cmf = sp.tile([B, 1], F32, name="cmfD")
            nc.gpsimd.tensor_copy(out=cmf[:], in_=cm8[:, 0:1])
            growf = sp.tile([B, 1], F32, name="growfD")
            nc.gpsimd.tensor_tensor(
                out=growf[:], in0=cmf[:], in1=brow[:], op=mybir.AluOpType.add
            )
            growf2 = sp.tile([B, 1], F32, name="growf2D")
            nc.gpsimd.tensor_scalar(
                out=growf2[:], in0=growf[:], scalar1=1.0,
                scalar2=float(CD * NCHUNK),
                op0=mybir.AluOpType.mult, op1=mybir.AluOpType.add,
            )
            growi = sp.tile([B, 1], I32, name="growiD")
            nc.gpsimd.tensor_copy(out=growi[:], in_=growf2[:])
            chunk = sp.tile([B, CHW], F32, name="chunkD")
            nc.gpsimd.indirect_dma_start(
                out=chunk[:], out_offset=None, in_=x_rows[:],
                in_offset=bass.IndirectOffsetOnAxis(ap=growi[:, 0:1], axis=0),
            )

            # thr / b_thr broadcast across batch partitions via PE
            pi8df = sp.tile([1, 8], F32, name="pi8fD")
            nc.gpsimd.tensor_copy(out=pi8df[:], in_=pi8d[:])
            duo = sp.tile([1, 2], F32, name="duoD")
            nc.gpsimd.tensor_copy(out=duo[:, 0:1], in_=pt8d[:, K - 1:K])
            nc.gpsimd.tensor_copy(out=duo[:, 1:2], in_=pi8df[:, K - 1:K])
            thr_ps = pp.tile([B, 2], F32, name="thrpsD")
            nc.tensor.matmul(
                out=thr_ps[:], lhsT=ones_row[:], rhs=duo[:],
                start=True, stop=True,
            )
            thrbc = sp.tile([B, 2], F32, name="thrbcD")
            nc.scalar.copy(out=thrbc[:], in_=thr_ps[:])

            # keep mask -> scatter row (OOB for non-survivors)
            # Pool engine has no compare ALU ops; these are tiny, Vector is
            # idle by now.
            gt = sp.tile([B, 1], F32, name="gtD")
            nc.vector.tensor_tensor(
                out=gt[:], in0=pooled[:, CD:CD + 1], in1=thrbc[:, 0:1],
                op=mybir.AluOpType.is_gt,
            )
            eq = sp.tile([B, 1], F32, name="eqD")
            nc.vector.tensor_tensor(
                out=eq[:], in0=pooled[:, CD:CD + 1], in1=thrbc[:, 0:1],
                op=mybir.AluOpType.is_equal,
            )
            ble = sp.tile([B, 1], F32, name="bleD")
            nc.vector.tensor_tensor(
                out=ble[:], in0=thrbc[:, 1:2], in1=iota_b[:],
                op=mybir.AluOpType.is_ge,
            )
            e2 = sp.tile([B, 1], F32, name="e2D")
            nc.gpsimd.tensor_tensor(
                out=e2[:], in0=eq[:], in1=ble[:], op=mybir.AluOpType.mult
            )
            keep = sp.tile([B, 1], F32, name="keepD")
            nc.gpsimd.tensor_tensor(
                out=keep[:], in0=gt[:], in1=e2[:], op=mybir.AluOpType.add
            )
            nk = sp.tile([B, 1], F32, name="nkD")
            nc.gpsimd.tensor_scalar(
                out=nk[:], in0=keep[:], scalar1=-BIG, scalar2=BIG,
                op0=mybir.AluOpType.mult, op1=mybir.AluOpType.add,
            )
            srowf = sp.tile([B, 1], F32, name="srowfD")
            nc.gpsimd.tensor_tensor(
                out=srowf[:], in0=growf2[:], in1=nk[:], op=mybir.AluOpType.add
            )
            srowi = sp.tile([B, 1], I32, name="srowiD")
            nc.gpsimd.tensor_copy(out=srowi[:], in_=srowf[:])

            # mask the chunk to {M at argmax, 0 elsewhere} on the Scalar engine
            s0 = sp.tile([B, CHW], F32, name="s0D")
            nc.scalar.activation(
                out=s0[:], in_=chunk[:],
                func=mybir.ActivationFunctionType.Identity,
                bias=pooled[:, CD:CD + 1], scale=-1.0,
            )
            ind = sp.tile([B, CHW], F32, name="indD")
            nc.scalar.activation(
                out=ind[:], in_=s0[:],
                func=mybir.ActivationFunctionType.Relu,
                bias=1.0, scale=-float(1 << 30),
            )
            mk = sp.tile([B, CHW], F32, name="mkD")
            nc.scalar.activation(
                out=mk[:], in_=ind[:],
                func=mybir.ActivationFunctionType.Identity,
                bias=0.0, scale=pooled[:, CD:CD + 1],
            )

            nc.gpsimd.indirect_dma_start(
                out=y_rows[:],
                out_offset=bass.IndirectOffsetOnAxis(ap=srowi[:, 0:1], axis=0),
                in_=mk[:], in_offset=None,
                bounds_check=n_rows - 1, oob_is_err=False,
            )

    nc.finalize()
    _nc_cache = nc
    return nc


def _install_profile_hook():
    """Inject the antenv.axon_hooks shim so trace=True captures NTFFs."""
    import sys
    import types

    if "antenv.axon_hooks" in sys.modules:
        return
    import antenv
    import trn_agent_boot.trn_boot as tb

    mod = types.ModuleType("antenv.axon_hooks")
    mod._hook = tb._ntff_profile_via_ctypes("/opt/axon/libaxon_pjrt.so")
    mod.get_axon_ntff_profile_hook = lambda: mod._hook
    mod.set_axon_ntff_profile_hook = lambda h: setattr(mod, "_hook", h)
    sys.modules["antenv.axon_hooks"] = mod
    antenv.axon_hooks = mod

    # no S3 in this container — keep artifacts local
    import concourse.bass_utils as bu

    bu.upload_artifacts = lambda tmpdir: tmpdir


def run(activations, trace=False):
    if trace:
        _install_profile_hook()
    act = np.asarray(activations)
    assert act.shape == (B, C_FULL, H, W), act.shape
    act = act.astype(np.float32, copy=False)
    nc = _build()
    in_maps = [
        {"x": np.ascontiguousarray(act[:, i * CSH:(i + 1) * CSH]).reshape(B, CSH, HW)}
        for i in range(N_CORES)
    ]
    res = run_bass_kernel_spmd(
        nc, in_maps, core_ids=list(range(N_CORES)), trace=trace
    )
    out = np.concatenate(
        [r["y"].reshape(B, CSH, H, W) for r in res.results], axis=1
    )
    return out, res


def kernel(activations):
    out, _ = run(activations, trace=False)
    return out


# revision 10
# speedup vs baseline: 1.0032x; 1.0002x over previous
"""CombinedSparsity (spatial max-pool + lifetime top-k + max-unpool) on 8 TRN2 cores.

Shard the 128 channels across 8 cores (16 each). Per (b, c) map the output is
all zeros except (possibly) one element: the map's max, written back at its
argmax position, kept only if that max is among the top-6 over the batch for
its channel. Output buffers are donated zero-filled, so each core only writes
the surviving elements.

Per channel (streamed one 2MB map-block at a time):
  - the streaming reduce keeps 64 chunk maxima (same DVE cost as a flat max),
  - a small FIND_INDEX8 over the 64 chunk maxima locates the (first) chunk
    that contains the map's max — exact argmax chunk per (b, c),
  - the winning 64-element chunk of every batch entry is re-gathered by one
    indirect DMA (32KB per channel instead of the full 2MB).
Per unit of 4 channels:
  - top-8 over the batch per channel via InstMax/InstMaxIndex on the PE-
    transposed pooled matrix (progressive duplicate handling keeps exact-tie
    batches identical to jax.lax.top_k),
  - survivor selection is a one-hot matrix built on the Scalar engine from the
    PE-broadcast top-6 batch indices; a per-channel PE matmul compacts
    [chunk | scatter_row | max] for the 6 survivors into column slices of one
    PSUM tile; one more PE transpose yields [24, 66],
  - three Scalar activation passes mask each chunk to {max at argmax, 0},
  - one indirect DMA scatters the 24 masked 256B rows into the output.
Vector only runs the streaming reduces plus tiny per-channel/unit ops, so it
never stalls on DMA round trips; all glue runs on GpSimd/Scalar/PE.
"""
import numpy as np

import concourse.bass as bass
import concourse.bacc as bacc
import concourse.tile as tile
from concourse import mybir
from concourse.bass_utils import run_bass_kernel_spmd
from concourse.masks import make_identity

B = 128
C_FULL = 128
H = 64
W = 64
HW = H * W
N_CORES = 8
CSH = C_FULL // N_CORES      # channels per core
K = 6                        # lifetime top-k
NCHUNK = 64                  # chunk maxima kept per map
CHW = HW // NCHUNK           # elements per chunk
EXT = CHW + 2                # chunk | scatter row | max
F32 = mybir.dt.float32
I32 = mybir.dt.int32
U32 = mybir.dt.uint32
UNITS = [(0, 4), (4, 8), (8, 12), (12, 16)]

_nc_cache = None


def _build():
    global _nc_cache
    if _nc_cache is not None:
        return _nc_cache

    nc = bacc.Bacc("TRN2", target_bir_lowering=False, debug=False)
    x = nc.dram_tensor("x", [B, CSH, HW], F32, kind="ExternalInput")
    y = nc.dram_tensor("y", [B, CSH, HW], F32, kind="ExternalOutput")
    x_rows = x.rearrange("b c (k j) -> (b c k) j", j=CHW)
    y_rows = y.rearrange("b c (k j) -> (b c k) j", j=CHW)

    with tile.TileContext(nc) as tc:
        with (
            tc.tile_pool(name="const", bufs=1) as cp,
            tc.tile_pool(name="gxp", bufs=5) as gxp,
            tc.tile_pool(name="pmp", bufs=4) as pmp,
            tc.tile_pool(name="small", bufs=1) as sp,
            tc.tile_pool(name="ps", bufs=1, space="PSUM") as pp,
        ):
            ident0 = cp.tile([B, B], F32)
            make_identity(nc, ident0[:])
            ident = cp.tile([B, B], F32)
            nc.vector.tensor_copy(out=ident[:], in_=ident0[:])
            ones_row = cp.tile([1, B], F32)
            nc.gpsimd.memset(ones_row[:], 1.0)
            iota_b_i = cp.tile([B, 1], I32)
            nc.gpsimd.iota(iota_b_i[:], pattern=[[1, 1]], base=0,
                           channel_multiplier=1)
            iota_b = cp.tile([B, 1], F32)
            nc.gpsimd.tensor_copy(out=iota_b[:], in_=iota_b_i[:])
            # b*CSH*NCHUNK: row of (b, c=0, chunk=0) in x_rows/y_rows
            brow = cp.tile([B, 1], F32)
            nc.gpsimd.tensor_scalar(
                out=brow[:], in0=iota_b[:], scalar1=float(CSH * NCHUNK),
                scalar2=None, op0=mybir.AluOpType.mult,
            )

            pooled = sp.tile([B, CSH], F32, name="pooled")
            pm = [None] * CSH
            ext = [
                sp.tile([B, EXT], F32, name=f"ext{c}") for c in range(CSH)
            ]
            oh_all = [None] * len(UNITS)

            def emit_channel_head(c):
                """stream DMA + chunked max reduce + pooled column."""
                gx = gxp.tile([B, HW], F32, name=f"gx{c}", tag="gx")
                pm[c] = pmp.tile([B, NCHUNK], F32, name=f"pm{c}", tag="pm")
                if c in (0, CSH - 1):
                    # split halves: first reduce starts sooner (c=0) /
                    # last reduce ends sooner after stream end (c=15)
                    hw2, nk2 = HW // 2, NCHUNK // 2
                    for hf in range(2):
                        nc.sync.dma_start(
                            out=gx[:, hf * hw2:(hf + 1) * hw2],
                            in_=x[:, c, hf * hw2:(hf + 1) * hw2],
                        )
                        nc.vector.tensor_reduce(
                            out=pm[c][:, hf * nk2:(hf + 1) * nk2],
                            in_=gx[:, hf * hw2:(hf + 1) * hw2].rearrange(
                                "p (k j) -> p k j", k=nk2),
                            axis=mybir.AxisListType.X,
                            op=mybir.AluOpType.max,
                        )
                else:
                    nc.sync.dma_start(out=gx[:], in_=x[:, c, :])
                    nc.vector.tensor_reduce(
                        out=pm[c][:],
                        in_=gx[:].rearrange("p (k j) -> p k j", k=NCHUNK),
                        axis=mybir.AxisListType.X,
                        op=mybir.AluOpType.max,
                    )
                nc.vector.tensor_reduce(
                    out=pooled[:, c:c + 1], in_=pm[c][:],
                    axis=mybir.AxisListType.X, op=mybir.AluOpType.max,
                )
                pbc8 = sp.tile([B, 8], F32, name=f"pbc8_{c}")
                nc.gpsimd.tensor_copy(
                    out=pbc8[:], in_=pooled[:, c:c + 1].to_broadcast([B, 8])
                )
                return pbc8

            def emit_channel_tail(c, pbc8):
                """argmax chunk + indirect gather of the winning chunks."""
                cm8 = sp.tile([B, 8], U32, name=f"cm8_{c}")
                nc.vector.max_index(out=cm8[:], in_max=pbc8[:],
                                    in_values=pm[c][:])
                cmf = sp.tile([B, 1], F32, name=f"cmf{c}")
                nc.gpsimd.tensor_copy(out=cmf[:], in_=cm8[:, 0:1])
                # grow = b*CSH*NCHUNK + c*NCHUNK + cm
                g1 = sp.tile([B, 1], F32, name=f"g1_{c}")
                nc.gpsimd.tensor_scalar(
                    out=g1[:], in0=cmf[:], scalar1=1.0,
                    scalar2=brow[:, 0:1],
                    op0=mybir.AluOpType.mult, op1=mybir.AluOpType.add,
                )
                nc.gpsimd.tensor_scalar(
                    out=ext[c][:, CHW:CHW + 1], in0=g1[:], scalar1=1.0,
                    scalar2=float(c * NCHUNK),
                    op0=mybir.AluOpType.mult, op1=mybir.AluOpType.add,
                )
                nc.gpsimd.tensor_copy(
                    out=ext[c][:, CHW + 1:CHW + 2], in_=pooled[:, c:c + 1]
                )
                growi = sp.tile([B, 1], I32, name=f"growi{c}")
                nc.gpsimd.tensor_copy(out=growi[:],
                                      in_=ext[c][:, CHW:CHW + 1])
                nc.gpsimd.indirect_dma_start(
                    out=ext[c][:, 0:CHW], out_offset=None, in_=x_rows[:],
                    in_offset=bass.IndirectOffsetOnAxis(
                        ap=growi[:, 0:1], axis=0
                    ),
                )

            def emit_topk(u):
                """transpose + top-8 + one-hot survivor columns."""
                c_lo, c_hi = UNITS[u]
                ncha = c_hi - c_lo
                pooled_t_ps = pp.tile([ncha, B], F32, name=f"ptps{u}",
                                      tag="pt")
                nc.tensor.transpose(
                    out=pooled_t_ps[:], in_=pooled[:, c_lo:c_hi],
                    identity=ident[:],
                )
                pooled_t = sp.tile([ncha, B], F32, name=f"pt{u}")
                nc.scalar.copy(out=pooled_t[:], in_=pooled_t_ps[:])
                pt8 = sp.tile([ncha, 8], F32, name=f"pt8{u}")
                nc.vector.max(out=pt8[:], in_=pooled_t[:])
                pi8 = sp.tile([ncha, 8], U32, name=f"pi8{u}")
                nc.vector.max_index(out=pi8[:], in_max=pt8[:],
                                    in_values=pooled_t[:])
                pi8f = sp.tile([ncha, 8], F32, name=f"pi8f{u}")
                nc.gpsimd.tensor_copy(out=pi8f[:], in_=pi8[:])
                # flatten the top-6 batch indices onto partition 0:
                # pivec[0, ci*K+j] = pi8f[ci, j] via identity-column selects
                pivec_ps = pp.tile([1, ncha * K], F32, name=f"pvps{u}",
                                   tag="pvps")
                for ci in range(ncha):
                    nc.tensor.matmul(
                        out=pivec_ps[:, ci * K:(ci + 1) * K],
                        lhsT=ident[0:ncha, ci:ci + 1], rhs=pi8f[:, 0:K],
                        start=True, stop=True,
                    )
                pivec = sp.tile([1, ncha * K], F32, name=f"pv{u}")
                nc.scalar.copy(out=pivec[:], in_=pivec_ps[:])
                ohbc = pp.tile([B, ncha * K], F32, name=f"ohbc{u}",
                               tag="ohbc")
                nc.tensor.matmul(
                    out=ohbc[:], lhsT=ones_row[:], rhs=pivec[:],
                    start=True, stop=True,
                )
                s1 = sp.tile([B, ncha * K], F32, name=f"s1_{u}")
                nc.scalar.activation(
                    out=s1[:], in_=ohbc[:],
                    func=mybir.ActivationFunctionType.Identity,
                    bias=iota_b[:, 0:1], scale=-1.0,
                )
                s2 = sp.tile([B, ncha * K], F32, name=f"s2_{u}")
                nc.scalar.activation(
                    out=s2[:], in_=s1[:],
                    func=mybir.ActivationFunctionType.Square,
                )
                oh_all[u] = sp.tile([B, ncha * K], F32, name=f"oha{u}")
                nc.scalar.activation(
                    out=oh_all[u][:], in_=s2[:],
                    func=mybir.ActivationFunctionType.Relu,
                    bias=1.0, scale=-1.0,
                )

            def emit_merge(u):
                """compact survivors, mask chunks, scatter rows."""
                c_lo, c_hi = UNITS[u]
                ncha = c_hi - c_lo
                nsurv = ncha * K
                cpsT = pp.tile([EXT, nsurv], F32, name=f"cpsT{u}", tag="cpsT")
                for ci in range(ncha):
                    c = c_lo + ci
                    nc.tensor.matmul(
                        out=cpsT[:, ci * K:(ci + 1) * K], lhsT=ext[c][:],
                        rhs=oh_all[u][:, ci * K:(ci + 1) * K],
                        start=True, stop=True,
                    )
                cpsT_sb = sp.tile([EXT, nsurv], F32, name=f"cpsTs{u}")
                nc.scalar.copy(out=cpsT_sb[:], in_=cpsT[:])
                cps = pp.tile([nsurv, EXT], F32, name=f"cps{u}", tag="cps")
                nc.tensor.transpose(
                    out=cps[:], in_=cpsT_sb[:], identity=ident[0:EXT, 0:EXT]
                )
                ce = sp.tile([nsurv, EXT], F32, name=f"ce{u}")
                nc.scalar.copy(out=ce[:], in_=cps[:])
                # mask chunk to {M at positions == M, 0 elsewhere}
                s0 = sp.tile([nsurv, CHW], F32, name=f"ms0_{u}")
                nc.scalar.activation(
                    out=s0[:], in_=ce[:, 0:CHW],
                    func=mybir.ActivationFunctionType.Identity,
                    bias=ce[:, CHW + 1:CHW + 2], scale=-1.0,
                )
                ind = sp.tile([nsurv, CHW], F32, name=f"mind{u}")
                nc.scalar.activation(
                    out=ind[:], in_=s0[:],
                    func=mybir.ActivationFunctionType.Relu,
                    bias=1.0, scale=-float(1 << 30),
                )
                mkv = sp.tile([nsurv, CHW], F32, name=f"mkv{u}")
                nc.scalar.activation(
                    out=mkv[:], in_=ind[:],
                    func=mybir.ActivationFunctionType.Identity,
                    bias=0.0, scale=ce[:, CHW + 1:CHW + 2],
                )
                offs = sp.tile([nsurv, 1], I32, name=f"offs{u}")
                nc.gpsimd.tensor_copy(out=offs[:], in_=ce[:, CHW:CHW + 1])
                nc.gpsimd.indirect_dma_start(
                    out=y_rows[:],
                    out_offset=bass.IndirectOffsetOnAxis(
                        ap=offs[:, 0:1], axis=0
                    ),
                    in_=mkv[:], in_offset=None,
                )

            # ---- emission: stream with per-channel tails one behind, unit
            # ---- top-k at each boundary, merges one unit later.
            pbc = [None] * CSH
            for c in range(CSH):
                pbc[c] = emit_channel_head(c)
                if c >= 1:
                    emit_channel_tail(c - 1, pbc[c - 1])
                if c == 4:
                    emit_topk(0)
                if c == 6:
                    emit_merge(0)
                if c == 8:
                    emit_topk(1)
                if c == 10:
                    emit_merge(1)
                if c == 12:
                    emit_topk(2)
                if c == 14:
                    emit_merge(2)
            emit_channel_tail(CSH - 1, pbc[CSH - 1])
            emit_topk(3)
            emit_merge(3)

    nc.finalize()
    _nc_cache = nc
    return nc


def _install_profile_hook():
    """Inject the antenv.axon_hooks shim so trace=True captures NTFFs."""
    import sys
    import types

    if "antenv.axon_hooks" in sys.modules:
        return
    import antenv
    import trn_agent_boot.trn_boot as tb

    mod = types.ModuleType("antenv.axon_hooks")
    mod._hook = tb._ntff_profile_via_ctypes("/opt/axon/libaxon_pjrt.so")
    mod.get_axon_ntff_profile_hook = lambda: mod._hook
    mod.set_axon_ntff_profile_hook = lambda h: setattr(mod, "_hook", h)
    sys.modules["antenv.axon_hooks"] = mod
    antenv.axon_hooks = mod

    # no S3 in this container — keep artifacts local
    import concourse.bass_utils as bu

    bu.upload_artifacts = lambda tmpdir: tmpdir


def run(activations, trace=False):
    if trace:
        _install_profile_hook()
    act = np.asarray(activations)
    assert act.shape == (B, C_FULL, H, W), act.shape
    act = act.astype(np.float32, copy=False)
    nc = _build()
    in_maps = [
        {"x": np.ascontiguousarray(act[:, i * CSH:(i + 1) * CSH]).reshape(B, CSH, HW)}
        for i in range(N_CORES)
    ]
    res = run_bass_kernel_spmd(
        nc, in_maps, core_ids=list(range(N_CORES)), trace=trace
    )
    out = np.concatenate(
        [r["y"].reshape(B, CSH, H, W) for r in res.results], axis=1
    )
    return out, res


def kernel(activations):
    out, _ = run(activations, trace=False)
    return out


# revision 12
# speedup vs baseline: 1.0476x; 1.0442x over previous
"""CombinedSparsity (spatial max-pool + lifetime top-k + max-unpool) on 8 TRN2 cores.

Shard the 128 channels across 8 cores (16 each). Per (b, c) map the output is
all zeros except (possibly) one element: the map's max, written back at its
argmax position, kept only if that max is among the top-6 over the batch for
its channel. Output buffers are donated zero-filled, so each core only writes
the surviving elements.

Per channel (streamed one 2MB map-block at a time):
  - the streaming reduce keeps 16 chunk maxima (same DVE cost as a flat max),
  - a small FIND_INDEX8 over the chunk maxima locates the (first) chunk that
    contains the map's max — the exact argmax chunk of every (b, c) map
    without a second full pass.
Per unit of 4 channels:
  - top-8 over the batch per channel via InstMax/InstMaxIndex on the PE-
    transposed pooled matrix (progressive duplicate handling keeps exact-tie
    batches identical to jax.lax.top_k),
  - a one-hot survivor matrix is built on the Scalar engine from the PE-
    broadcast top-6 batch indices; per-channel PE matmuls compact each
    survivor's [chunk row | max] pair; one more PE transpose yields [24, 2],
  - ONE indirect DMA gathers the 24 surviving 1KB chunks (24 descriptors --
    indirect-DMA descriptor generation steals bandwidth from DMA engine E79,
    so survivors are compacted BEFORE gathering, not after),
  - three Scalar activation passes mask each chunk to {max at argmax, 0},
  - one indirect DMA scatters the 24 masked rows into the output.
Vector only runs the streaming reduces plus tiny per-channel/unit ops, so it
never stalls on DMA round trips; all glue runs on GpSimd/Scalar/PE. Stream
DMA triggers rotate across Sync/Scalar/Tensor queues so startup descriptor
generation pipelines.
"""
import numpy as np

import concourse.bass as bass
import concourse.bacc as bacc
import concourse.tile as tile
from concourse import mybir
from concourse.bass_utils import run_bass_kernel_spmd
from concourse.masks import make_identity

B = 128
C_FULL = 128
H = 64
W = 64
HW = H * W
N_CORES = 8
CSH = C_FULL // N_CORES      # channels per core
K = 6                        # lifetime top-k
NCHUNK = 16                  # chunk maxima kept per map
CHW = HW // NCHUNK           # elements per chunk
F32 = mybir.dt.float32
I32 = mybir.dt.int32
U32 = mybir.dt.uint32
UNITS = [(0, 4), (4, 8), (8, 12), (12, 16)]

_nc_cache = None


def _build():
    global _nc_cache
    if _nc_cache is not None:
        return _nc_cache

    nc = bacc.Bacc("TRN2", target_bir_lowering=False, debug=False)
    x = nc.dram_tensor("x", [B, CSH, HW], F32, kind="ExternalInput")
    y = nc.dram_tensor("y", [B, CSH, HW], F32, kind="ExternalOutput")
    x_rows = x.rearrange("b c (k j) -> (b c k) j", j=CHW)
    y_rows = y.rearrange("b c (k j) -> (b c k) j", j=CHW)

    with tile.TileContext(nc) as tc:
        with (
            tc.tile_pool(name="const", bufs=1) as cp,
            tc.tile_pool(name="gxp", bufs=5) as gxp,
            tc.tile_pool(name="pmp", bufs=4) as pmp,
            tc.tile_pool(name="small", bufs=1) as sp,
            tc.tile_pool(name="ps", bufs=1, space="PSUM") as pp,
        ):
            ident0 = cp.tile([B, B], F32)
            make_identity(nc, ident0[:])
            ident = cp.tile([B, B], F32)
            nc.vector.tensor_copy(out=ident[:], in_=ident0[:])
            ones_row = cp.tile([1, B], F32)
            nc.gpsimd.memset(ones_row[:], 1.0)
            iota_b_i = cp.tile([B, 1], I32)
            nc.gpsimd.iota(iota_b_i[:], pattern=[[1, 1]], base=0,
                           channel_multiplier=1)
            iota_b = cp.tile([B, 1], F32)
            nc.gpsimd.tensor_copy(out=iota_b[:], in_=iota_b_i[:])
            # b*CSH*NCHUNK: row of (b, c=0, chunk=0) in x_rows/y_rows
            brow = cp.tile([B, 1], F32)
            nc.gpsimd.tensor_scalar(
                out=brow[:], in0=iota_b[:], scalar1=float(CSH * NCHUNK),
                scalar2=None, op0=mybir.AluOpType.mult,
            )

            pooled = sp.tile([B, CSH], F32, name="pooled")
            pm = [None] * CSH
            # ext2[c] = [chunk row | max] per batch entry
            ext2 = [
                sp.tile([B, 2], F32, name=f"ext{c}") for c in range(CSH)
            ]
            oh_all = [None] * len(UNITS)
            trig = [nc.sync, nc.scalar]

            def emit_channel_head(c):
                """stream DMA + chunked max reduce + pooled column."""
                gx = gxp.tile([B, HW], F32, name=f"gx{c}", tag="gx")
                pm[c] = pmp.tile([B, NCHUNK], F32, name=f"pm{c}", tag="pm")
                eng = trig[c % len(trig)]
                if c == CSH - 1:
                    # split halves: the last reduce ends sooner after the
                    # stream ends
                    hw2, nk2 = HW // 2, NCHUNK // 2
                    for hf in range(2):
                        eng.dma_start(
                            out=gx[:, hf * hw2:(hf + 1) * hw2],
                            in_=x[:, c, hf * hw2:(hf + 1) * hw2],
                        )
                        nc.vector.tensor_reduce(
                            out=pm[c][:, hf * nk2:(hf + 1) * nk2],
                            in_=gx[:, hf * hw2:(hf + 1) * hw2].rearrange(
                                "p (k j) -> p k j", k=nk2),
                            axis=mybir.AxisListType.X,
                            op=mybir.AluOpType.max,
                        )
                else:
                    eng.dma_start(out=gx[:], in_=x[:, c, :])
                    nc.vector.tensor_reduce(
                        out=pm[c][:],
                        in_=gx[:].rearrange("p (k j) -> p k j", k=NCHUNK),
                        axis=mybir.AxisListType.X,
                        op=mybir.AluOpType.max,
                    )
                nc.vector.tensor_reduce(
                    out=pooled[:, c:c + 1], in_=pm[c][:],
                    axis=mybir.AxisListType.X, op=mybir.AluOpType.max,
                )
                pbc8 = sp.tile([B, 8], F32, name=f"pbc8_{c}")
                nc.gpsimd.tensor_copy(
                    out=pbc8[:], in_=pooled[:, c:c + 1].to_broadcast([B, 8])
                )
                return pbc8

            def emit_channel_tail(c, pbc8):
                """argmax chunk of each map -> ext2 = [chunk row | max]."""
                cm8 = sp.tile([B, 8], U32, name=f"cm8_{c}")
                nc.vector.max_index(out=cm8[:], in_max=pbc8[:],
                                    in_values=pm[c][:])
                cmf = sp.tile([B, 1], F32, name=f"cmf{c}")
                nc.gpsimd.tensor_copy(out=cmf[:], in_=cm8[:, 0:1])
                g1 = sp.tile([B, 1], F32, name=f"g1_{c}")
                nc.gpsimd.tensor_scalar(
                    out=g1[:], in0=cmf[:], scalar1=1.0,
                    scalar2=brow[:, 0:1],
                    op0=mybir.AluOpType.mult, op1=mybir.AluOpType.add,
                )
                nc.gpsimd.tensor_scalar(
                    out=ext2[c][:, 0:1], in0=g1[:], scalar1=1.0,
                    scalar2=float(c * NCHUNK),
                    op0=mybir.AluOpType.mult, op1=mybir.AluOpType.add,
                )
                nc.gpsimd.tensor_copy(
                    out=ext2[c][:, 1:2], in_=pooled[:, c:c + 1]
                )

            def emit_topk(u):
                """transpose + top-8 + one-hot survivor columns."""
                c_lo, c_hi = UNITS[u]
                ncha = c_hi - c_lo
                pooled_t_ps = pp.tile([ncha, B], F32, name=f"ptps{u}",
                                      tag="pt")
                nc.tensor.transpose(
                    out=pooled_t_ps[:], in_=pooled[:, c_lo:c_hi],
                    identity=ident[:],
                )
                pooled_t = sp.tile([ncha, B], F32, name=f"pt{u}")
                nc.scalar.copy(out=pooled_t[:], in_=pooled_t_ps[:])
                pt8 = sp.tile([ncha, 8], F32, name=f"pt8{u}")
                nc.vector.max(out=pt8[:], in_=pooled_t[:])
                pi8 = sp.tile([ncha, 8], U32, name=f"pi8{u}")
                nc.vector.max_index(out=pi8[:], in_max=pt8[:],
                                    in_values=pooled_t[:])
                pi8f = sp.tile([ncha, 8], F32, name=f"pi8f{u}")
                nc.gpsimd.tensor_copy(out=pi8f[:], in_=pi8[:])
                # flatten the top-6 batch indices onto partition 0:
                # pivec[0, ci*K+j] = pi8f[ci, j] via identity-column selects
                pivec_ps = pp.tile([1, ncha * K], F32, name=f"pvps{u}",
                                   tag="pvps")
                for ci in range(ncha):
                    nc.tensor.matmul(
                        out=pivec_ps[:, ci * K:(ci + 1) * K],
                        lhsT=ident[0:ncha, ci:ci + 1], rhs=pi8f[:, 0:K],
                        start=True, stop=True,
                    )
                pivec = sp.tile([1, ncha * K], F32, name=f"pv{u}")
                nc.scalar.copy(out=pivec[:], in_=pivec_ps[:])
                ohbc = pp.tile([B, ncha * K], F32, name=f"ohbc{u}",
                               tag="ohbc")
                nc.tensor.matmul(
                    out=ohbc[:], lhsT=ones_row[:], rhs=pivec[:],
                    start=True, stop=True,
                )
                s1 = sp.tile([B, ncha * K], F32, name=f"s1_{u}")
                nc.scalar.activation(
                    out=s1[:], in_=ohbc[:],
                    func=mybir.ActivationFunctionType.Identity,
                    bias=iota_b[:, 0:1], scale=-1.0,
                )
                s2 = sp.tile([B, ncha * K], F32, name=f"s2_{u}")
                nc.scalar.activation(
                    out=s2[:], in_=s1[:],
                    func=mybir.ActivationFunctionType.Square,
                )
                oh_all[u] = sp.tile([B, ncha * K], F32, name=f"oha{u}")
                nc.scalar.activation(
                    out=oh_all[u][:], in_=s2[:],
                    func=mybir.ActivationFunctionType.Relu,
                    bias=1.0, scale=-1.0,
                )

            def emit_merge(u):
                """compact survivors, gather chunks, mask, scatter."""
                c_lo, c_hi = UNITS[u]
                ncha = c_hi - c_lo
                nsurv = ncha * K
                cpsT = pp.tile([2, nsurv], F32, name=f"cpsT{u}", tag="cpsT")
                for ci in range(ncha):
                    c = c_lo + ci
                    nc.tensor.matmul(
                        out=cpsT[:, ci * K:(ci + 1) * K], lhsT=ext2[c][:],
                        rhs=oh_all[u][:, ci * K:(ci + 1) * K],
                        start=True, stop=True,
                    )
                cpsT_sb = sp.tile([2, nsurv], F32, name=f"cpsTs{u}")
                nc.scalar.copy(out=cpsT_sb[:], in_=cpsT[:])
                cps = pp.tile([nsurv, 2], F32, name=f"cps{u}", tag="cps")
                nc.tensor.transpose(
                    out=cps[:], in_=cpsT_sb[:], identity=ident[0:2, 0:2]
                )
                ce = sp.tile([nsurv, 2], F32, name=f"ce{u}")
                nc.scalar.copy(out=ce[:], in_=cps[:])
                rows_i = sp.tile([nsurv, 1], I32, name=f"rows{u}")
                nc.gpsimd.tensor_copy(out=rows_i[:], in_=ce[:, 0:1])
                # gather the surviving chunks
                cx = sp.tile([nsurv, CHW], F32, name=f"cx{u}")
                nc.gpsimd.indirect_dma_start(
                    out=cx[:], out_offset=None, in_=x_rows[:],
                    in_offset=bass.IndirectOffsetOnAxis(
                        ap=rows_i[:, 0:1], axis=0
                    ),
                )
                # mask each chunk to {M at positions == M, 0 elsewhere}
                s0 = sp.tile([nsurv, CHW], F32, name=f"ms0_{u}")
                nc.scalar.activation(
                    out=s0[:], in_=cx[:],
                    func=mybir.ActivationFunctionType.Identity,
                    bias=ce[:, 1:2], scale=-1.0,
                )
                ind = sp.tile([nsurv, CHW], F32, name=f"mind{u}")
                nc.scalar.activation(
                    out=ind[:], in_=s0[:],
                    func=mybir.ActivationFunctionType.Relu,
                    bias=1.0, scale=-float(1 << 30),
                )
                mkv = sp.tile([nsurv, CHW], F32, name=f"mkv{u}")
                nc.scalar.activation(
                    out=mkv[:], in_=ind[:],
                    func=mybir.ActivationFunctionType.Identity,
                    bias=0.0, scale=ce[:, 1:2],
                )
                nc.gpsimd.indirect_dma_start(
                    out=y_rows[:],
                    out_offset=bass.IndirectOffsetOnAxis(
                        ap=rows_i[:, 0:1], axis=0
                    ),
                    in_=mkv[:], in_offset=None,
                )

            # ---- emission: stream with per-channel tails one behind, unit
            # ---- top-k + merge one channel after each unit boundary.
            pbc = [None] * CSH
            for c in range(CSH):
                pbc[c] = emit_channel_head(c)
                if c >= 1:
                    emit_channel_tail(c - 1, pbc[c - 1])
                if c in (5, 9, 13):
                    u = c // 4 - 1
                    emit_topk(u)
                    emit_merge(u)
            emit_channel_tail(CSH - 1, pbc[CSH - 1])
            emit_topk(3)
            emit_merge(3)

    nc.finalize()
    _nc_cache = nc
    return nc


def _install_profile_hook():
    """Inject the antenv.axon_hooks shim so trace=True captures NTFFs."""
    import sys
    import types

    if "antenv.axon_hooks" in sys.modules:
        return
    import antenv
    import trn_agent_boot.trn_boot as tb

    mod = types.ModuleType("antenv.axon_hooks")
    mod._hook = tb._ntff_profile_via_ctypes("/opt/axon/libaxon_pjrt.so")
    mod.get_axon_ntff_profile_hook = lambda: mod._hook
    mod.set_axon_ntff_profile_hook = lambda h: setattr(mod, "_hook", h)
    sys.modules["antenv.axon_hooks"] = mod
    antenv.axon_hooks = mod

    # no S3 in this container — keep artifacts local
    import concourse.bass_utils as bu

    bu.upload_artifacts = lambda tmpdir: tmpdir


def run(activations, trace=False):
    if trace:
        _install_profile_hook()
    act = np.asarray(activations)
    assert act.shape == (B, C_FULL, H, W), act.shape
    act = act.astype(np.float32, copy=False)
    nc = _build()
    in_maps = [
        {"x": np.ascontiguousarray(act[:, i * CSH:(i + 1) * CSH]).reshape(B, CSH, HW)}
        for i in range(N_CORES)
    ]
    res = run_bass_kernel_spmd(
        nc, in_maps, core_ids=list(range(N_CORES)), trace=trace
    )
    out = np.concatenate(
        [r["y"].reshape(B, CSH, H, W) for r in res.results], axis=1
    )
    return out, res


def kernel(activations):
    out, _ = run(activations, trace=False)
    return out
